# revision 5
# baseline (speedup 1.0000x reference)
"""Trainium2 Bass kernel for nn_Attention_82540681494971.

Spatial self-attention block (LDM AttnBlock style, unscaled):
  qkv = conv1x1(x);  s = q^T k  [n x n] per (b,head);  attn = softmax(s, axis=-1)
  out[d,m] = sum_n v[d,n] attn[n,m];  y = conv1x1(out)

Shapes: B=4, C=64, H=W=64 -> n=4096 tokens, HEAD=4, d=16.

Sharding: 8 cores, core c handles batch b=c//2 and heads (0,1) if c%2==0
else (2,3). Each core computes a partial projection output over its two
heads' channels; host sums the two partials per batch and adds proj bias.

Key algebra: attn[n,m] = E[n,m]/rowsum[n] with E=exp(s). Since the AV
contraction runs over n (the softmax row index), fold 1/rowsum into v:
  out[d,m] = sum_n (v[d,n]*rinv[n]) E[n,m]
so the big E matrix never needs normalizing. Scores are exact-fp32-grade
via a 3-term bf16 split (q=q_hi+q_lo, k=k_hi+k_lo, drop lo*lo):
  s = [q_hi;q_lo;q_hi]^T [k_hi;k_hi;k_lo]   (K=48 stacked, 1 cyc/row)
"""

import numpy as np
from contextlib import ExitStack

import concourse.bass as bass
import concourse.mybir as mybir
import concourse.tile as tile
from concourse import bacc
from concourse.bass import ts, ds
from concourse.bass_utils import run_bass_kernel_spmd

F32 = mybir.dt.float32
BF16 = mybir.dt.bfloat16
AF = mybir.ActivationFunctionType

B, C, HEAD, D = 4, 64, 4, 16
N = 4096          # tokens = H*W
NT = 128          # n-tile (partition) size
NTILES = N // NT  # 32
MC = 512          # matmul free-dim chunk
SCH = (1536, 1536, 1024)  # scores/exp PSUM chunking

E_DT = BF16       # dtype of exp(s) and v~ for the AV matmul
G = 8 if E_DT == BF16 else 4   # n-tiles per AV accumulation group


def _body(tc, y, x1, wq, wk, wv, wp0, wp1):
    nc = tc.nc
    ctx = ExitStack()
    with ctx:
        pp = ctx.enter_context(tc.tile_pool(name="persist", bufs=1))
        cp = ctx.enter_context(tc.tile_pool(name="consts", bufs=1))
        sp = ctx.enter_context(tc.tile_pool(name="spsum", bufs=2, space="PSUM"))
        ap = ctx.enter_context(tc.tile_pool(name="avpsum", bufs=2, space="PSUM"))

        # ---- constants ----
        wq_t = cp.tile([C + 1, 2 * D], F32)
        wk_t = cp.tile([C + 1, 2 * D], F32)
        wv_t = cp.tile([C + 1, 2 * D], F32)
        wp0_t = cp.tile([D, C], F32)
        wp1_t = cp.tile([D, C], F32)
        nc.sync.dma_start(wq_t[:], wq[:])
        nc.sync.dma_start(wk_t[:], wk[:])
        nc.sync.dma_start(wv_t[:], wv[:])
        nc.sync.dma_start(wp0_t[:], wp0[:])
        nc.sync.dma_start(wp1_t[:], wp1[:])

        # ---- persistent SBUF ----
        qsp = pp.tile([3 * D, 2 * N], BF16)  # [q_hi; q_lo; q_hi], head-major cols
        ksp = pp.tile([3 * D, 2 * N], BF16)  # [k_hi; k_hi; k_lo]
        vT_sb = pp.tile([NT, NTILES * 2 * D], F32)  # per n-tile: [128, 32] (h0|h1)
        out_h = [pp.tile([D, N], F32, tag=f"out{h}", name=f"out{h}")
                 for h in range(2)]
        y_sb = pp.tile([C, N], F32)

        # ---- phase 0: qkv + bf16 hi/lo split ----
        with (
            tc.tile_pool(name="x1p", bufs=1) as xp,
            tc.tile_pool(name="qkf", bufs=1) as qf,
            tc.tile_pool(name="spl", bufs=2) as spl,
        ):
            x1_t = xp.tile([C + 1, N], F32)
            nc.sync.dma_start(x1_t[:], x1[:])

            for which, w_t, dst, dup in (
                ("q", wq_t, qsp, 2),   # dup row-block 2 gets hi
                ("k", wk_t, ksp, 1),   # dup row-block 1 gets hi
            ):
                f32_sb = qf.tile([D, 2 * N], F32, tag="qkf32")
                for h in range(2):
                    for mc in range(N // MC):
                        ps = ap.tile([D, MC], F32, tag="av")
                        nc.tensor.matmul(
                            ps[:], w_t[:, ts(h, D)], x1_t[:, ts(mc, MC)],
                            start=True, stop=True)
                        nc.vector.tensor_copy(
                            f32_sb[:, ds(h * N + mc * MC, MC)], ps[:])
                hi_t = spl.tile([D, 2 * N], BF16, tag="hi")
                lo_t = spl.tile([D, 2 * N], BF16, tag="lo")
                nc.vector.tensor_copy(hi_t[:], f32_sb[:])
                nc.vector.tensor_sub(lo_t[:], f32_sb[:], hi_t[:])
                # assemble K=48 stack via SBUF->SBUF DMA (no partition
                # alignment limits on DMA)
                nc.sync.dma_start(dst[0:D, :], hi_t[:])
                lo_block = 1 if which == "q" else 2
                nc.sync.dma_start(dst[ds(lo_block * D, D), :], lo_t[:])
                nc.sync.dma_start(dst[ds(dup * D, D), :], hi_t[:])

            for nt in range(NTILES):
                psv = ap.tile([NT, 2 * D], F32, tag="av")
                nc.tensor.matmul(
                    psv[:], x1_t[:, ts(nt, NT)], wv_t[:],
                    start=True, stop=True)
                nc.vector.tensor_copy(vT_sb[:, ts(nt, 2 * D)], psv[:])

        # ---- phase 1: attention per head ----
        with (
            tc.tile_pool(name="ep", bufs=G + 2) as ep,
            tc.tile_pool(name="rp", bufs=4) as rp,
            tc.tile_pool(name="vp", bufs=G + 2) as vp,
        ):
            for h in range(2):
                for g in range(NTILES // G):
                    e_tiles, vts_tiles = [], []
                    for j in range(G):
                        nt = g * G + j
                        e_t = ep.tile([NT, N], E_DT, tag="e")
                        rsp = rp.tile([NT, 4], F32, tag="rs")
                        off = 0
                        for ci, csz in enumerate(SCH):
                            s_ps = sp.tile([NT, SCH[0]], F32, tag="s")
                            for i in range(csz // MC):
                                nc.tensor.matmul(
                                    s_ps[:, ts(i, MC)],
                                    qsp[:, ds(h * N + nt * NT, NT)],
                                    ksp[:, ds(h * N + off + i * MC, MC)],
                                    start=True, stop=True)
                            nc.scalar.activation(
                                e_t[:, ds(off, csz)], s_ps[:, :csz], AF.Exp,
                                accum_out=rsp[:, ds(ci, 1)])
                            off += csz
                        rs = rp.tile([NT, 1], F32, tag="r1")
                        rinv = rp.tile([NT, 1], F32, tag="ri")
                        nc.vector.reduce_sum(
                            rs[:], rsp[:, 0:3], axis=mybir.AxisListType.X)
                        nc.vector.reciprocal(rinv[:], rs[:])
                        vts = vp.tile([NT, D], E_DT, tag="vts")
                        nc.vector.tensor_scalar_mul(
                            vts[:], vT_sb[:, ds(nt * 2 * D + h * D, D)], rinv[:])
                        e_tiles.append(e_t)
                        vts_tiles.append(vts)

                    for mc in range(N // MC):
                        av = ap.tile([D, MC], F32, tag="av")
                        for j in range(G):
                            nc.tensor.matmul(
                                av[:], vts_tiles[j][:],
                                e_tiles[j][:, ts(mc, MC)],
                                start=(j == 0), stop=(j == G - 1))
                        dst = out_h[h][:, ts(mc, MC)]
                        if g == 0:
                            nc.vector.tensor_copy(dst, av[:])
                        else:
                            nc.vector.tensor_add(dst, dst, av[:])

        # ---- phase 2: partial projection (bias added on host) ----
        for mc in range(N // MC):
            yp = ap.tile([C, MC], F32, tag="av")
            nc.tensor.matmul(yp[:], wp0_t[:], out_h[0][:, ts(mc, MC)],
                             start=True, stop=False)
            nc.tensor.matmul(yp[:], wp1_t[:], out_h[1][:, ts(mc, MC)],
                             start=False, stop=True)
            nc.vector.tensor_copy(y_sb[:, ts(mc, MC)], yp[:])
        nc.sync.dma_start(y[:], y_sb[:])


_PROGRAM = None


def _get_program():
    global _PROGRAM
    if _PROGRAM is None:
        nc = bacc.Bacc("TRN2", target_bir_lowering=False, debug=False,
                       num_devices=8)
        x1 = nc.dram_tensor("x1", [C + 1, N], F32, kind="ExternalInput").ap()
        wq = nc.dram_tensor("wq", [C + 1, 2 * D], F32, kind="ExternalInput").ap()
        wk = nc.dram_tensor("wk", [C + 1, 2 * D], F32, kind="ExternalInput").ap()
        wv = nc.dram_tensor("wv", [C + 1, 2 * D], F32, kind="ExternalInput").ap()
        wp0 = nc.dram_tensor("wp0", [D, C], F32, kind="ExternalInput").ap()
        wp1 = nc.dram_tensor("wp1", [D, C], F32, kind="ExternalInput").ap()
        y = nc.dram_tensor("y", [C, N], F32, kind="ExternalOutput").ap()
        with tile.TileContext(nc) as tc:
            _body(tc, y, x1, wq, wk, wv, wp0, wp1)
        nc.compile()
        _PROGRAM = nc
    return _PROGRAM


def _make_in_maps(x, qkv_w, qkv_b, proj_w, proj_b=None):
    x = np.asarray(x, dtype=np.float32)
    qkv_w = np.asarray(qkv_w, dtype=np.float32)
    qkv_b = np.asarray(qkv_b, dtype=np.float32)
    proj_w = np.asarray(proj_w, dtype=np.float32)

    in_maps = []
    for core in range(8):
        b = core // 2
        h0 = 2 * (core % 2)
        heads = (h0, h0 + 1)
        x1 = np.concatenate(
            [x[b].reshape(C, N), np.ones((1, N), np.float32)], axis=0)

        def aug_qk(block):
            w = np.empty((C + 1, 2 * D), np.float32)
            for j, h in enumerate(heads):
                rows = slice(block * C + h * D, block * C + (h + 1) * D)
                w[:C, j * D:(j + 1) * D] = qkv_w[rows, :].T
                w[C, j * D:(j + 1) * D] = qkv_b[rows]
            return w

        wp_parts = [
            np.ascontiguousarray(proj_w[:, h * D:(h + 1) * D].T)
            for h in heads
        ]

        in_maps.append({
            "x1": np.ascontiguousarray(x1),
            "wq": aug_qk(0),
            "wk": aug_qk(1),
            "wv": aug_qk(2),
            "wp0": wp_parts[0],
            "wp1": wp_parts[1],
        })
    return in_maps


def run_cores(inputs, **kw):
    """Compile+run on the 8 cores; returns BassKernelResults."""
    nc = _get_program()
    in_maps = _make_in_maps(**inputs)
    return run_bass_kernel_spmd(nc, in_maps, list(range(8)), **kw)


def kernel(x, qkv_w, qkv_b, proj_w, proj_b):
    res = run_cores(dict(x=x, qkv_w=qkv_w, qkv_b=qkv_b,
                         proj_w=proj_w, proj_b=proj_b))
    proj_b = np.asarray(proj_b, dtype=np.float32)
    parts = [r["y"] for r in res.results]
    out = np.empty((B, C, N), np.float32)
    for b in range(B):
        out[b] = parts[2 * b] + parts[2 * b + 1] + proj_b[:, None]
    return out.reshape(B, C, 64, 64)


if __name__ == "__main__":
    _get_program()
    print("program built OK")


# revision 7
# speedup vs baseline: 1.0920x; 1.0920x over previous
"""Trainium2 Bass kernel for nn_Attention_82540681494971.

Spatial self-attention block (LDM AttnBlock style, unscaled):
  qkv = conv1x1(x);  s = q^T k  [n x n] per (b,head);  attn = softmax(s, axis=-1)
  out[d,m] = sum_n v[d,n] attn[n,m];  y = conv1x1(out)

Shapes: B=4, C=64, H=W=64 -> n=4096 tokens, HEAD=4, d=16.

Sharding: 8 cores, core c handles batch b=c//2 and heads (0,1) if c%2==0
else (2,3). Each core computes a partial projection output over its two
heads' channels; host sums the two partials per batch and adds proj bias.

Key algebra: attn[n,m] = E[n,m]/rowsum[n] with E=exp(s). Since the AV
contraction runs over n (the softmax row index), fold 1/rowsum into v:
  out[d,m] = sum_n (v[d,n]*rinv[n]) E[n,m]
so the big E matrix never needs normalizing. Scores are exact-fp32-grade
via a 3-term bf16 split (q=q_hi+q_lo, k=k_hi+k_lo, drop lo*lo):
  s = [q_hi;q_lo;q_hi]^T [k_hi;k_hi;k_lo]   (K=48 stacked, 1 cyc/row)
"""

import numpy as np
from contextlib import ExitStack

import concourse.bass as bass
import concourse.mybir as mybir
import concourse.tile as tile
from concourse import bacc
from concourse.bass import ts, ds
from concourse.bass_utils import run_bass_kernel_spmd

F32 = mybir.dt.float32
BF16 = mybir.dt.bfloat16
AF = mybir.ActivationFunctionType

B, C, HEAD, D = 4, 64, 4, 16
N = 4096          # tokens = H*W
NT = 128          # n-tile (partition) size
NTILES = N // NT  # 32
MC = 512          # matmul free-dim chunk
SCH = (1536, 1536, 1024)  # scores/exp PSUM chunking

E_DT = BF16       # dtype of exp(s) and v~ for the AV matmul
G = 4             # n-tiles per AV accumulation group


def _body(tc, y, x1, wq, wk, wv, wp0, wp1):
    nc = tc.nc
    ctx = ExitStack()
    with ctx:
        pp = ctx.enter_context(tc.tile_pool(name="persist", bufs=1))
        cp = ctx.enter_context(tc.tile_pool(name="consts", bufs=1))
        sp = ctx.enter_context(tc.tile_pool(name="spsum", bufs=2, space="PSUM"))
        ap = ctx.enter_context(tc.tile_pool(name="avpsum", bufs=2, space="PSUM"))

        # ---- constants ----
        wq_t = cp.tile([C + 1, 2 * D], F32)
        wk_t = cp.tile([C + 1, 2 * D], F32)
        wv_t = cp.tile([C + 1, 2 * D], F32)
        wp0_t = cp.tile([D, C], F32)
        wp1_t = cp.tile([D, C], F32)
        nc.sync.dma_start(wq_t[:], wq[:])
        nc.sync.dma_start(wk_t[:], wk[:])
        nc.sync.dma_start(wv_t[:], wv[:])
        nc.sync.dma_start(wp0_t[:], wp0[:])
        nc.sync.dma_start(wp1_t[:], wp1[:])

        # ---- persistent SBUF ----
        qsp = pp.tile([3 * D, 2 * N], BF16)  # [q_hi; q_lo; q_hi], head-major cols
        ksp = pp.tile([3 * D, 2 * N], BF16)  # [k_hi; k_hi; k_lo]
        vT_sb = pp.tile([NT, NTILES * 2 * D], F32)  # per n-tile: [128, 32] (h0|h1)
        out_h = [pp.tile([D, N], F32, tag=f"out{h}", name=f"out{h}")
                 for h in range(2)]
        y_sb = pp.tile([C, N], F32)

        # ---- phase 0: qkv + bf16 hi/lo split ----
        with (
            tc.tile_pool(name="x1p", bufs=1) as xp,
            tc.tile_pool(name="qkf", bufs=1) as qf,
            tc.tile_pool(name="spl", bufs=2) as spl,
        ):
            x1_t = xp.tile([C + 1, N], F32)
            nc.sync.dma_start(x1_t[:], x1[:])

            for which, w_t, dst, dup in (
                ("q", wq_t, qsp, 2),   # dup row-block 2 gets hi
                ("k", wk_t, ksp, 1),   # dup row-block 1 gets hi
            ):
                f32_sb = qf.tile([D, 2 * N], F32, tag="qkf32")
                for h in range(2):
                    for mc in range(N // MC):
                        ps = ap.tile([D, MC], F32, tag="av")
                        nc.tensor.matmul(
                            ps[:], w_t[:, ts(h, D)], x1_t[:, ts(mc, MC)],
                            start=True, stop=True)
                        nc.vector.tensor_copy(
                            f32_sb[:, ds(h * N + mc * MC, MC)], ps[:])
                hi_t = spl.tile([D, 2 * N], BF16, tag="hi")
                lo_t = spl.tile([D, 2 * N], BF16, tag="lo")
                nc.vector.tensor_copy(hi_t[:], f32_sb[:])
                nc.vector.tensor_sub(lo_t[:], f32_sb[:], hi_t[:])
                # assemble K=48 stack via SBUF->SBUF DMA (no partition
                # alignment limits on DMA)
                nc.sync.dma_start(dst[0:D, :], hi_t[:])
                lo_block = 1 if which == "q" else 2
                nc.sync.dma_start(dst[ds(lo_block * D, D), :], lo_t[:])
                nc.sync.dma_start(dst[ds(dup * D, D), :], hi_t[:])

            for nt in range(NTILES):
                psv = ap.tile([NT, 2 * D], F32, tag="av")
                nc.tensor.matmul(
                    psv[:], x1_t[:, ts(nt, NT)], wv_t[:],
                    start=True, stop=True)
                nc.vector.tensor_copy(vT_sb[:, ts(nt, 2 * D)], psv[:])

        # ---- phase 1: attention, software-pipelined ----
        # Per step (one n-tile): emit the scores matmuls + exp for this
        # n-tile AND the AV chains of the *previous* group, so the PE queue
        # interleaves score matmuls with AV matmuls. This keeps the PE busy
        # (and HAM-warm) while ACT streams exp, and keeps ACT fed while the
        # PE runs AV chains.
        CHAINS_PER_STEP = (N // MC) // G  # AV chains emitted per step

        def av_chain(ph, pg, pv, pe, mc):
            av = ap.tile([D, MC], F32, tag="av", name=f"av{ph}_{pg}_{mc}")
            for j in range(G):
                nc.tensor.matmul(
                    av[:], pv[j][:], pe[j][:, ts(mc, MC)],
                    start=(j == 0), stop=(j == G - 1))
            dst = out_h[ph][:, ts(mc, MC)]
            if pg == 0:
                nc.vector.tensor_copy(dst, av[:])
            else:
                nc.vector.tensor_add(dst, dst, av[:])

        with (
            tc.tile_pool(name="ep", bufs=2 * G + 2) as ep,
            tc.tile_pool(name="rp", bufs=4) as rp,
            tc.tile_pool(name="vp", bufs=2 * G + 2) as vp,
        ):
            prev = None  # (head, group, vts_tiles, e_tiles)
            for h in range(2):
                for g in range(NTILES // G):
                    e_tiles, vts_tiles = [], []
                    for j in range(G):
                        nt = g * G + j
                        e_t = ep.tile([NT, N], E_DT, tag="e",
                                      name=f"e{h}_{nt}")
                        rsp = rp.tile([NT, 4], F32, tag="rs", name="rsp")
                        off = 0
                        for ci, csz in enumerate(SCH):
                            s_ps = sp.tile([NT, SCH[0]], F32, tag="s",
                                           name="s_ps")
                            for i in range(csz // MC):
                                nc.tensor.matmul(
                                    s_ps[:, ts(i, MC)],
                                    qsp[:, ds(h * N + nt * NT, NT)],
                                    ksp[:, ds(h * N + off + i * MC, MC)],
                                    start=True, stop=True)
                            nc.scalar.activation(
                                e_t[:, ds(off, csz)], s_ps[:, :csz], AF.Exp,
                                accum_out=rsp[:, ds(ci, 1)])
                            off += csz
                        rs = rp.tile([NT, 1], F32, tag="r1", name="rs")
                        rinv = rp.tile([NT, 1], F32, tag="ri", name="rinv")
                        nc.vector.reduce_sum(
                            rs[:], rsp[:, 0:3], axis=mybir.AxisListType.X)
                        nc.vector.reciprocal(rinv[:], rs[:])
                        vts = vp.tile([NT, D], E_DT, tag="vts",
                                      name=f"vts{h}_{nt}")
                        nc.vector.tensor_scalar_mul(
                            vts[:], vT_sb[:, ds(nt * 2 * D + h * D, D)], rinv[:])
                        e_tiles.append(e_t)
                        vts_tiles.append(vts)
                        if prev is not None:
                            ph, pg, pv, pe = prev
                            for c in range(CHAINS_PER_STEP):
                                av_chain(ph, pg, pv, pe,
                                         j * CHAINS_PER_STEP + c)
                    prev = (h, g, vts_tiles, e_tiles)
            ph, pg, pv, pe = prev
            for mc in range(N // MC):
                av_chain(ph, pg, pv, pe, mc)

        # ---- phase 2: partial projection (bias added on host) ----
        for mc in range(N // MC):
            yp = ap.tile([C, MC], F32, tag="av")
            nc.tensor.matmul(yp[:], wp0_t[:], out_h[0][:, ts(mc, MC)],
                             start=True, stop=False)
            nc.tensor.matmul(yp[:], wp1_t[:], out_h[1][:, ts(mc, MC)],
                             start=False, stop=True)
            nc.vector.tensor_copy(y_sb[:, ts(mc, MC)], yp[:])
        nc.sync.dma_start(y[:], y_sb[:])


_PROGRAM = None


def _get_program():
    global _PROGRAM
    if _PROGRAM is None:
        nc = bacc.Bacc("TRN2", target_bir_lowering=False, debug=False,
                       num_devices=8)
        x1 = nc.dram_tensor("x1", [C + 1, N], F32, kind="ExternalInput").ap()
        wq = nc.dram_tensor("wq", [C + 1, 2 * D], F32, kind="ExternalInput").ap()
        wk = nc.dram_tensor("wk", [C + 1, 2 * D], F32, kind="ExternalInput").ap()
        wv = nc.dram_tensor("wv", [C + 1, 2 * D], F32, kind="ExternalInput").ap()
        wp0 = nc.dram_tensor("wp0", [D, C], F32, kind="ExternalInput").ap()
        wp1 = nc.dram_tensor("wp1", [D, C], F32, kind="ExternalInput").ap()
        y = nc.dram_tensor("y", [C, N], F32, kind="ExternalOutput").ap()
        with tile.TileContext(nc) as tc:
            _body(tc, y, x1, wq, wk, wv, wp0, wp1)
        nc.compile()
        _PROGRAM = nc
    return _PROGRAM


def _make_in_maps(x, qkv_w, qkv_b, proj_w, proj_b=None):
    x = np.asarray(x, dtype=np.float32)
    qkv_w = np.asarray(qkv_w, dtype=np.float32)
    qkv_b = np.asarray(qkv_b, dtype=np.float32)
    proj_w = np.asarray(proj_w, dtype=np.float32)

    in_maps = []
    for core in range(8):
        b = core // 2
        h0 = 2 * (core % 2)
        heads = (h0, h0 + 1)
        x1 = np.concatenate(
            [x[b].reshape(C, N), np.ones((1, N), np.float32)], axis=0)

        def aug_qk(block):
            w = np.empty((C + 1, 2 * D), np.float32)
            for j, h in enumerate(heads):
                rows = slice(block * C + h * D, block * C + (h + 1) * D)
                w[:C, j * D:(j + 1) * D] = qkv_w[rows, :].T
                w[C, j * D:(j + 1) * D] = qkv_b[rows]
            return w

        wp_parts = [
            np.ascontiguousarray(proj_w[:, h * D:(h + 1) * D].T)
            for h in heads
        ]

        in_maps.append({
            "x1": np.ascontiguousarray(x1),
            "wq": aug_qk(0),
            "wk": aug_qk(1),
            "wv": aug_qk(2),
            "wp0": wp_parts[0],
            "wp1": wp_parts[1],
        })
    return in_maps


def run_cores(inputs, **kw):
    """Compile+run on the 8 cores; returns BassKernelResults."""
    nc = _get_program()
    in_maps = _make_in_maps(**inputs)
    return run_bass_kernel_spmd(nc, in_maps, list(range(8)), **kw)


def kernel(x, qkv_w, qkv_b, proj_w, proj_b):
    res = run_cores(dict(x=x, qkv_w=qkv_w, qkv_b=qkv_b,
                         proj_w=proj_w, proj_b=proj_b))
    proj_b = np.asarray(proj_b, dtype=np.float32)
    parts = [r["y"] for r in res.results]
    out = np.empty((B, C, N), np.float32)
    for b in range(B):
        out[b] = parts[2 * b] + parts[2 * b + 1] + proj_b[:, None]
    return out.reshape(B, C, 64, 64)


if __name__ == "__main__":
    _get_program()
    print("program built OK")


# revision 12
# speedup vs baseline: 1.0935x; 1.0013x over previous
"""Trainium2 Bass kernel for nn_Attention_82540681494971.

Spatial self-attention block (LDM AttnBlock style, unscaled):
  qkv = conv1x1(x);  s = q^T k  [n x n] per (b,head);  attn = softmax(s, axis=-1)
  out[d,m] = sum_n v[d,n] attn[n,m];  y = conv1x1(out)

Shapes: B=4, C=64, H=W=64 -> n=4096 tokens, HEAD=4, d=16.

Sharding: 8 cores, core c handles batch b=c//2 and heads (0,1) if c%2==0
else (2,3). Each core computes a partial projection output over its two
heads' channels; host sums the two partials per batch and adds proj bias.

Key algebra: attn[n,m] = E[n,m]/rowsum[n] with E=exp(s). Since the AV
contraction runs over n (the softmax row index), fold 1/rowsum into v:
  out[d,m] = sum_n (v[d,n]*rinv[n]) E[n,m]
so the big E matrix never needs normalizing. Scores are exact-fp32-grade
via a 3-term bf16 split (q=q_hi+q_lo, k=k_hi+k_lo, drop lo*lo):
  s = [q_hi;q_lo;q_hi]^T [k_hi;k_hi;k_lo]   (K=48 stacked, 1 cyc/row)
"""

import numpy as np
from contextlib import ExitStack

import concourse.bass as bass
import concourse.mybir as mybir
import concourse.tile as tile
from concourse import bacc
from concourse.bass import ts, ds
from concourse.bass_utils import run_bass_kernel_spmd

F32 = mybir.dt.float32
BF16 = mybir.dt.bfloat16
AF = mybir.ActivationFunctionType

B, C, HEAD, D = 4, 64, 4, 16
N = 4096          # tokens = H*W
NT = 128          # n-tile (partition) size
NTILES = N // NT  # 32
MC = 512          # matmul free-dim chunk
SCH = (1536, 1536, 1024)  # scores/exp PSUM chunking

E_DT = BF16       # dtype of exp(s) and v~ for the AV matmul
G = 4             # n-tiles per AV accumulation group


def _body(tc, y, x1, wq, wk, wv, wp0, wp1):
    nc = tc.nc
    ctx = ExitStack()
    with ctx:
        pp = ctx.enter_context(tc.tile_pool(name="persist", bufs=1))
        cp = ctx.enter_context(tc.tile_pool(name="consts", bufs=1))
        sp = ctx.enter_context(tc.tile_pool(name="spsum", bufs=2, space="PSUM"))
        ap = ctx.enter_context(tc.tile_pool(name="avpsum", bufs=2, space="PSUM"))

        # ---- constants ----
        wq_t = cp.tile([C + 1, 2 * D], F32)
        wk_t = cp.tile([C + 1, 2 * D], F32)
        wv_t = cp.tile([C + 1, 2 * D], F32)
        wp0_t = cp.tile([D, C], F32)
        wp1_t = cp.tile([D, C], F32)
        nc.sync.dma_start(wq_t[:], wq[:])
        nc.sync.dma_start(wk_t[:], wk[:])
        nc.sync.dma_start(wv_t[:], wv[:])
        nc.sync.dma_start(wp0_t[:], wp0[:])
        nc.sync.dma_start(wp1_t[:], wp1[:])

        # ---- persistent SBUF ----
        qsp = pp.tile([3 * D, 2 * N], BF16)  # [q_hi; q_lo; q_hi], head-major cols
        ksp = pp.tile([3 * D, 2 * N], BF16)  # [k_hi; k_hi; k_lo]
        vT_sb = pp.tile([NT, NTILES * 2 * D], F32)  # per n-tile: [128, 32] (h0|h1)
        out_h = [pp.tile([D, N], F32, tag=f"out{h}", name=f"out{h}")
                 for h in range(2)]
        y_sb = pp.tile([C, N], F32)

        # ---- phase 0: qkv + bf16 hi/lo split ----
        with (
            tc.tile_pool(name="x1p", bufs=1) as xp,
            tc.tile_pool(name="qkf", bufs=1) as qf,
            tc.tile_pool(name="spl", bufs=2) as spl,
        ):
            x1_t = xp.tile([C + 1, N], F32)
            for i in range(8):  # parallel DMA queues
                nc.sync.dma_start(x1_t[:, ts(i, N // 8)], x1[:, ts(i, N // 8)])

            for which, w_t, dst, dup in (
                ("q", wq_t, qsp, 2),   # dup row-block 2 gets hi
                ("k", wk_t, ksp, 1),   # dup row-block 1 gets hi
            ):
                f32_sb = qf.tile([D, 2 * N], F32, tag="qkf32")
                for h in range(2):
                    for mc in range(N // MC):
                        ps = ap.tile([D, MC], F32, tag="av")
                        nc.tensor.matmul(
                            ps[:], w_t[:, ts(h, D)], x1_t[:, ts(mc, MC)],
                            start=True, stop=True)
                        # alternate evacuation engine to halve critical path
                        dst_ap = f32_sb[:, ds(h * N + mc * MC, MC)]
                        if mc % 2 == 0:
                            nc.vector.tensor_copy(dst_ap, ps[:])
                        else:
                            nc.scalar.copy(dst_ap, ps[:])
                hi_t = spl.tile([D, 2 * N], BF16, tag="hi")
                lo_t = spl.tile([D, 2 * N], BF16, tag="lo")
                # hi-cast on ACT (idle in phase 0), lo-sub on DVE; halves
                nc.scalar.copy(hi_t[:, 0:N], f32_sb[:, 0:N])
                nc.scalar.copy(hi_t[:, N:2 * N], f32_sb[:, N:2 * N])
                nc.vector.tensor_sub(lo_t[:, 0:N], f32_sb[:, 0:N],
                                     hi_t[:, 0:N])
                nc.vector.tensor_sub(lo_t[:, N:2 * N], f32_sb[:, N:2 * N],
                                     hi_t[:, N:2 * N])
                # assemble K=48 stack via SBUF->SBUF DMA (no partition
                # alignment limits on DMA); chunked for queue parallelism
                lo_block = 1 if which == "q" else 2
                for i in range(4):
                    sl = ts(i, N // 2)
                    nc.sync.dma_start(dst[0:D, sl], hi_t[:, sl])
                    nc.sync.dma_start(dst[ds(lo_block * D, D), sl],
                                      lo_t[:, sl])
                    nc.sync.dma_start(dst[ds(dup * D, D), sl], hi_t[:, sl])

            for nt in range(NTILES):
                psv = ap.tile([NT, 2 * D], F32, tag="av")
                nc.tensor.matmul(
                    psv[:], x1_t[:, ts(nt, NT)], wv_t[:],
                    start=True, stop=True)
                if nt % 2 == 0:
                    nc.vector.tensor_copy(vT_sb[:, ts(nt, 2 * D)], psv[:])
                else:
                    nc.scalar.copy(vT_sb[:, ts(nt, 2 * D)], psv[:])

        # ---- phase 1: attention, software-pipelined ----
        # Per step (one n-tile): emit the scores matmuls + exp for this
        # n-tile AND the AV chains of the *previous* group, so the PE queue
        # interleaves score matmuls with AV matmuls. This keeps the PE busy
        # (and HAM-warm) while ACT streams exp, and keeps ACT fed while the
        # PE runs AV chains.
        CHAINS_PER_STEP = (N // MC) // G  # AV chains emitted per step

        def av_chain(ph, pg, pv, pe, mc):
            av = ap.tile([D, MC], F32, tag="av", name=f"av{ph}_{pg}_{mc}")
            for j in range(G):
                nc.tensor.matmul(
                    av[:], pv[j][:], pe[j][:, ts(mc, MC)],
                    start=(j == 0), stop=(j == G - 1))
            dst = out_h[ph][:, ts(mc, MC)]
            if pg == 0:
                nc.vector.tensor_copy(dst, av[:])
            else:
                nc.vector.tensor_add(dst, dst, av[:])

        with (
            tc.tile_pool(name="ep", bufs=2 * G + 2) as ep,
            tc.tile_pool(name="rp", bufs=4) as rp,
            tc.tile_pool(name="vp", bufs=2 * G + 2) as vp,
        ):
            prev = None  # (head, group, vts_tiles, e_tiles)
            for h in range(2):
                for g in range(NTILES // G):
                    e_tiles, vts_tiles = [], []
                    for j in range(G):
                        nt = g * G + j
                        e_t = ep.tile([NT, N], E_DT, tag="e",
                                      name=f"e{h}_{nt}")
                        rsp = rp.tile([NT, 4], F32, tag="rs", name="rsp")
                        off = 0
                        for ci, csz in enumerate(SCH):
                            s_ps = sp.tile([NT, SCH[0]], F32, tag="s",
                                           name="s_ps")
                            for i in range(csz // MC):
                                nc.tensor.matmul(
                                    s_ps[:, ts(i, MC)],
                                    qsp[:, ds(h * N + nt * NT, NT)],
                                    ksp[:, ds(h * N + off + i * MC, MC)],
                                    start=True, stop=True)
                            nc.scalar.activation(
                                e_t[:, ds(off, csz)], s_ps[:, :csz], AF.Exp,
                                accum_out=rsp[:, ds(ci, 1)])
                            off += csz
                        rs = rp.tile([NT, 1], F32, tag="r1", name="rs")
                        rinv = rp.tile([NT, 1], F32, tag="ri", name="rinv")
                        nc.vector.reduce_sum(
                            rs[:], rsp[:, 0:3], axis=mybir.AxisListType.X)
                        nc.vector.reciprocal(rinv[:], rs[:])
                        vts = vp.tile([NT, D], E_DT, tag="vts",
                                      name=f"vts{h}_{nt}")
                        nc.vector.tensor_scalar_mul(
                            vts[:], vT_sb[:, ds(nt * 2 * D + h * D, D)], rinv[:])
                        e_tiles.append(e_t)
                        vts_tiles.append(vts)
                        if prev is not None:
                            ph, pg, pv, pe = prev
                            for c in range(CHAINS_PER_STEP):
                                av_chain(ph, pg, pv, pe,
                                         j * CHAINS_PER_STEP + c)
                    prev = (h, g, vts_tiles, e_tiles)
            # ---- tail: flush last group's chains, interleaved with the
            # projection (proj for chunk mc can start once chain mc lands)
            ph, pg, pv, pe = prev
            for mc in range(N // MC):
                av_chain(ph, pg, pv, pe, mc)
                yp = ap.tile([C, MC], F32, tag="av", name=f"yp{mc}")
                nc.tensor.matmul(yp[:], wp0_t[:], out_h[0][:, ts(mc, MC)],
                                 start=True, stop=False)
                nc.tensor.matmul(yp[:], wp1_t[:], out_h[1][:, ts(mc, MC)],
                                 start=False, stop=True)
                if mc % 2 == 0:
                    nc.vector.tensor_copy(y_sb[:, ts(mc, MC)], yp[:])
                else:
                    nc.scalar.copy(y_sb[:, ts(mc, MC)], yp[:])
                nc.sync.dma_start(y[:, ts(mc, MC)], y_sb[:, ts(mc, MC)])


_PROGRAM = None


def _get_program():
    global _PROGRAM
    if _PROGRAM is None:
        nc = bacc.Bacc("TRN2", target_bir_lowering=False, debug=False,
                       num_devices=8)
        x1 = nc.dram_tensor("x1", [C + 1, N], F32, kind="ExternalInput").ap()
        wq = nc.dram_tensor("wq", [C + 1, 2 * D], F32, kind="ExternalInput").ap()
        wk = nc.dram_tensor("wk", [C + 1, 2 * D], F32, kind="ExternalInput").ap()
        wv = nc.dram_tensor("wv", [C + 1, 2 * D], F32, kind="ExternalInput").ap()
        wp0 = nc.dram_tensor("wp0", [D, C], F32, kind="ExternalInput").ap()
        wp1 = nc.dram_tensor("wp1", [D, C], F32, kind="ExternalInput").ap()
        y = nc.dram_tensor("y", [C, N], F32, kind="ExternalOutput").ap()
        with tile.TileContext(nc) as tc:
            _body(tc, y, x1, wq, wk, wv, wp0, wp1)
        nc.compile()
        _PROGRAM = nc
    return _PROGRAM


def _make_in_maps(x, qkv_w, qkv_b, proj_w, proj_b=None):
    x = np.asarray(x, dtype=np.float32)
    qkv_w = np.asarray(qkv_w, dtype=np.float32)
    qkv_b = np.asarray(qkv_b, dtype=np.float32)
    proj_w = np.asarray(proj_w, dtype=np.float32)

    in_maps = []
    for core in range(8):
        b = core // 2
        h0 = 2 * (core % 2)
        heads = (h0, h0 + 1)
        x1 = np.concatenate(
            [x[b].reshape(C, N), np.ones((1, N), np.float32)], axis=0)

        def aug_qk(block):
            w = np.empty((C + 1, 2 * D), np.float32)
            for j, h in enumerate(heads):
                rows = slice(block * C + h * D, block * C + (h + 1) * D)
                w[:C, j * D:(j + 1) * D] = qkv_w[rows, :].T
                w[C, j * D:(j + 1) * D] = qkv_b[rows]
            return w

        wp_parts = [
            np.ascontiguousarray(proj_w[:, h * D:(h + 1) * D].T)
            for h in heads
        ]

        in_maps.append({
            "x1": np.ascontiguousarray(x1),
            "wq": aug_qk(0),
            "wk": aug_qk(1),
            "wv": aug_qk(2),
            "wp0": wp_parts[0],
            "wp1": wp_parts[1],
        })
    return in_maps


def run_cores(inputs, **kw):
    """Compile+run on the 8 cores; returns BassKernelResults."""
    nc = _get_program()
    in_maps = _make_in_maps(**inputs)
    return run_bass_kernel_spmd(nc, in_maps, list(range(8)), **kw)


def kernel(x, qkv_w, qkv_b, proj_w, proj_b):
    res = run_cores(dict(x=x, qkv_w=qkv_w, qkv_b=qkv_b,
                         proj_w=proj_w, proj_b=proj_b))
    proj_b = np.asarray(proj_b, dtype=np.float32)
    parts = [r["y"] for r in res.results]
    out = np.empty((B, C, N), np.float32)
    for b in range(B):
        out[b] = parts[2 * b] + parts[2 * b + 1] + proj_b[:, None]
    return out.reshape(B, C, 64, 64)


if __name__ == "__main__":
    _get_program()
    print("program built OK")
